# revision 5
# baseline (speedup 1.0000x reference)
"""ConvCapsules2d Trainium2 kernel (Bass/Tile), SPMD over 8 NeuronCores.

Full problem:
  poses (16,32,16,14,14) f32, W (32,32,16,3,3) f32
  V[n,b,c,d,f,g,k,l] = W[b,c,d,k,l] * sum_p poses[n,b,p,2f+k,2g+l]
  V: (16,32,32,16,6,6,3,3) f32  (~340 MB -> memory/write bound)

Sharding: data-parallel over batch N: core i computes n in [2i, 2i+2).

Per-core layout: SBUF partition q = n*64 + b*2 + clo (n in 0..1, b in 0..31,
clo in 0..1), with output channel c = 2*m + clo split into 16 c-pairs m.
Free dims carry (d, f, g, k, l) = 5184 contiguous elements. Each of the 32
output stores is a (64 partition x 5184) DMA whose DRAM access pattern is
[[165888,32],[5184,2],[1,5184]] — outer count 32 so the descriptor spray
engages the full SDMA engine set (the previous (n,clo,b) ordering balanced
to an outer count of 2, which ran at ~55 GB/s instead of ~400 GB/s).

W is pre-permuted on the host to (64, 2304) = (b*2+clo, m*144+d*9+kl) so the
whole weight load is two contiguous full-spray DMAs. The n-duplication of
poses across the two 64-partition halves comes from re-reading the small
poses input; no cross-partition traffic on chip.
"""
import numpy as np

import concourse.bacc as bacc
import concourse.mybir as mybir
from concourse.tile import TileContext
from concourse import bass_utils

# ---- problem constants (hardcoded per contest contract) ----
NTOT, B, P, H = 16, 32, 16, 14
C, D, K, S = 32, 16, 3, 2
F = (H - K) // S + 1          # 6
FF, KK = F * F, K * K         # 36, 9
NCORES = 8
N = NTOT // NCORES            # 2 batches per core
NPART = 128
M = C // 2                    # 16 c-pairs
UNIT = D * FF * KK            # 5184 elements per (partition, m)


def _build(nc):
    poses = nc.dram_tensor("poses", (N, B, P, H, H), mybir.dt.float32, kind="ExternalInput")
    Wt = nc.dram_tensor("W", (2 * B, M * D * KK), mybir.dt.float32, kind="ExternalInput")
    V = nc.dram_tensor("V", (N, B, C, D, F, F, K, K), mybir.dt.float32, kind="ExternalOutput")

    with TileContext(nc) as tc:
        with tc.tile_pool(name="const", bufs=1) as const_pool, \
             tc.tile_pool(name="work", bufs=2) as wpool, \
             tc.tile_pool(name="out", bufs=4) as opool:
            # ---- loads; partition q = n*64 + b*2 + clo
            poses_sb = const_pool.tile([NPART, P * H * H], mybir.dt.float32)
            # (n, b, pij) -> duplicate each b-row into the clo pair via a
            # stride-0 middle dim on the DMA source.
            pos_src = poses.ap().rearrange("n b p i j -> n b (p i j)")[:, :, None, :] \
                               .broadcast_to((N, B, 2, P * H * H))
            for n in range(N):
                nc.sync.dma_start(out=poses_sb[n * 64:(n + 1) * 64, :], in_=pos_src[n])

            W_sb = const_pool.tile([NPART, M * D * KK], mybir.dt.float32)
            for n in range(N):
                nc.sync.dma_start(out=W_sb[n * 64:(n + 1) * 64, :], in_=Wt.ap())

            # ---- s-phase: sum over P (binary tree of wide adds), then unfold
            HH = H * H
            acc = wpool.tile([NPART, HH], mybir.dt.float32, tag="acc")
            tmp = wpool.tile([NPART, HH * 8], mybir.dt.float32, tag="tmp")
            nc.vector.tensor_add(out=tmp[:, :HH * 8],
                                 in0=poses_sb[:, :HH * 8],
                                 in1=poses_sb[:, HH * 8:])
            nc.vector.tensor_add(out=tmp[:, :HH * 4],
                                 in0=tmp[:, :HH * 4],
                                 in1=tmp[:, HH * 4:HH * 8])
            nc.vector.tensor_add(out=tmp[:, :HH * 2],
                                 in0=tmp[:, :HH * 2],
                                 in1=tmp[:, HH * 2:HH * 4])
            nc.vector.tensor_add(out=acc[:], in0=tmp[:, :HH], in1=tmp[:, HH:HH * 2])

            s2 = wpool.tile([NPART, FF * KK], mybir.dt.float32, tag="s2")
            acc_v = acc[:].rearrange("q (i j) -> q i j", i=H)
            s2_v = s2[:].rearrange("q (f g k l) -> q f g k l", f=F, g=F, k=K)
            for k in range(K):
                for l in range(K):
                    nc.vector.tensor_copy(out=s2_v[:, :, :, k, l],
                                          in_=acc_v[:, k:k + 2 * F - 1:2, l:l + 2 * F - 1:2])

            # ---- multiply + store per c-pair m
            vap = V.ap().rearrange("n b (m clo) d f g k l -> n m b clo (d f g k l)", clo=2)
            w_all = W_sb[:].rearrange("q (m d kl) -> q m d kl", m=M, d=D)
            s_bc = s2[:].rearrange("q (fg kl) -> q fg kl", kl=KK)[:, None, :, :] \
                        .broadcast_to((NPART, D, FF, KK))
            for m in range(M):
                out_t = opool.tile([NPART, UNIT], mybir.dt.float32, tag="out")
                out_v = out_t[:].rearrange("q (d fg kl) -> q d fg kl", d=D, fg=FF)
                w_view = w_all[:, m, :, None, :].broadcast_to((NPART, D, FF, KK))
                nc.vector.tensor_mul(out=out_v, in0=w_view, in1=s_bc)
                for n in range(N):
                    nc.sync.dma_start(out=vap[n, m], in_=out_t[n * 64:(n + 1) * 64, :])
    return nc


def permute_W(W: np.ndarray) -> np.ndarray:
    """(B, C, D, K, K) -> (2B, M*D*KK): row b*2+clo holds W[b, 2m+clo, d, k, l]."""
    Wp = W.reshape(B, M, 2, D, KK).transpose(0, 2, 1, 3, 4).reshape(2 * B, M * D * KK)
    return np.ascontiguousarray(Wp)


_cached_nc = None


def _get_nc():
    global _cached_nc
    if _cached_nc is None:
        nc = bacc.Bacc("TRN2", target_bir_lowering=False)
        _build(nc)
        nc.compile()
        _cached_nc = nc
    return _cached_nc


def run_spmd(poses: np.ndarray, W: np.ndarray, **spmd_kwargs):
    """Shard, run on 8 cores, gather. Returns (V_full, BassKernelResults)."""
    poses = np.ascontiguousarray(np.asarray(poses, dtype=np.float32))
    W = np.ascontiguousarray(np.asarray(W, dtype=np.float32))
    assert poses.shape == (NTOT, B, P, H, H), poses.shape
    assert W.shape == (B, C, D, K, K), W.shape
    Wp = permute_W(W)
    nc = _get_nc()
    in_maps = [{"poses": poses[i * N:(i + 1) * N], "W": Wp} for i in range(NCORES)]
    res = bass_utils.run_bass_kernel_spmd(nc, in_maps, core_ids=list(range(NCORES)),
                                          **spmd_kwargs)
    V = np.concatenate([r["V"] for r in res.results], axis=0)
    return V, res


def kernel(poses: np.ndarray, W: np.ndarray) -> np.ndarray:
    import time
    last_err = None
    for attempt in range(3):
        try:
            V, _ = run_spmd(poses, W)
            return V
        except Exception as e:  # transient NRT/axon device errors: poke + retry
            last_err = e
            time.sleep(2.0)
            try:
                import jax, jax.numpy as jnp
                jnp.sum(jnp.ones((8, 8))).block_until_ready()
            except Exception:
                pass
    raise last_err


# revision 8
# speedup vs baseline: 1.0242x; 1.0242x over previous
"""ConvCapsules2d Trainium2 kernel (Bass/Tile), SPMD over 8 NeuronCores.

Full problem:
  poses (16,32,16,14,14) f32, W (32,32,16,3,3) f32
  V[n,b,c,d,f,g,k,l] = W[b,c,d,k,l] * sum_p poses[n,b,p,2f+k,2g+l]
  V: (16,32,32,16,6,6,3,3) f32  (~340 MB -> memory/write bound)

Sharding: data-parallel over batch N: core i computes n in [2i, 2i+2).

Per-core layout: SBUF partition q = n*64 + b*2 + clo (n in 0..1, b in 0..31,
clo in 0..1), with output channel c = 2*m + clo split into 16 c-pairs m.
Free dims carry (d, f, g, k, l) = 5184 contiguous elements. Each of the 32
output stores is a (64 partition x 5184) DMA whose DRAM access pattern is
[[165888,32],[5184,2],[1,5184]] — outer count 32 so the descriptor spray
engages the full SDMA engine set (the previous (n,clo,b) ordering balanced
to an outer count of 2, which ran at ~55 GB/s instead of ~400 GB/s).

W is pre-permuted on the host to (64, 2304) = (b*2+clo, m*144+d*9+kl) so the
whole weight load is two contiguous full-spray DMAs. The n-duplication of
poses across the two 64-partition halves comes from re-reading the small
poses input; no cross-partition traffic on chip.
"""
import numpy as np

import concourse.bacc as bacc
import concourse.mybir as mybir
from concourse.tile import TileContext
from concourse import bass_utils

# ---- problem constants (hardcoded per contest contract) ----
NTOT, B, P, H = 16, 32, 16, 14
C, D, K, S = 32, 16, 3, 2
F = (H - K) // S + 1          # 6
FF, KK = F * F, K * K         # 36, 9
NCORES = 8
N = NTOT // NCORES            # 2 batches per core
NPART = 128
M = C // 2                    # 16 c-pairs
UNIT = D * FF * KK            # 5184 elements per (partition, m)


def _build(nc):
    # Both inputs arrive host-pre-arranged as one (128, free) row per SBUF
    # partition (q = n*64 + b*2 + clo), so each load is a single contiguous
    # full-spray DMA.
    poses = nc.dram_tensor("poses", (NPART, P * H * H), mybir.dt.float32, kind="ExternalInput")
    Wt = nc.dram_tensor("W", (NPART, M * D * KK), mybir.dt.float32, kind="ExternalInput")
    V = nc.dram_tensor("V", (N, B, C, D, F, F, K, K), mybir.dt.float32, kind="ExternalOutput")

    with TileContext(nc) as tc:
        with tc.tile_pool(name="const", bufs=1) as const_pool, \
             tc.tile_pool(name="work", bufs=2) as wpool, \
             tc.tile_pool(name="out", bufs=4) as opool:
            # ---- loads; partition q = n*64 + b*2 + clo
            poses_sb = const_pool.tile([NPART, P * H * H], mybir.dt.float32)
            nc.sync.dma_start(out=poses_sb[:], in_=poses.ap())

            W_sb = const_pool.tile([NPART, M * D * KK], mybir.dt.float32)
            nc.sync.dma_start(out=W_sb[:], in_=Wt.ap())

            # ---- s-phase: sum over P (binary tree of wide adds), then unfold
            HH = H * H
            acc = wpool.tile([NPART, HH], mybir.dt.float32, tag="acc")
            tmp = wpool.tile([NPART, HH * 8], mybir.dt.float32, tag="tmp")
            nc.vector.tensor_add(out=tmp[:, :HH * 8],
                                 in0=poses_sb[:, :HH * 8],
                                 in1=poses_sb[:, HH * 8:])
            nc.vector.tensor_add(out=tmp[:, :HH * 4],
                                 in0=tmp[:, :HH * 4],
                                 in1=tmp[:, HH * 4:HH * 8])
            nc.vector.tensor_add(out=tmp[:, :HH * 2],
                                 in0=tmp[:, :HH * 2],
                                 in1=tmp[:, HH * 2:HH * 4])
            nc.vector.tensor_add(out=acc[:], in0=tmp[:, :HH], in1=tmp[:, HH:HH * 2])

            s2 = wpool.tile([NPART, FF * KK], mybir.dt.float32, tag="s2")
            acc_v = acc[:].rearrange("q (i j) -> q i j", i=H)
            s2_v = s2[:].rearrange("q (f g k l) -> q f g k l", f=F, g=F, k=K)
            for k in range(K):
                for l in range(K):
                    nc.vector.tensor_copy(out=s2_v[:, :, :, k, l],
                                          in_=acc_v[:, k:k + 2 * F - 1:2, l:l + 2 * F - 1:2])

            # ---- multiply + store per c-pair m
            vap = V.ap().rearrange("n b (m clo) d f g k l -> n m b clo (d f g k l)", clo=2)
            w_all = W_sb[:].rearrange("q (m d kl) -> q m d kl", m=M, d=D)
            s_bc = s2[:].rearrange("q (fg kl) -> q fg kl", kl=KK)[:, None, :, :] \
                        .broadcast_to((NPART, D, FF, KK))
            for m in range(M):
                out_t = opool.tile([NPART, UNIT], mybir.dt.float32, tag="out")
                out_v = out_t[:].rearrange("q (d fg kl) -> q d fg kl", d=D, fg=FF)
                w_view = w_all[:, m, :, None, :].broadcast_to((NPART, D, FF, KK))
                nc.vector.tensor_mul(out=out_v, in0=w_view, in1=s_bc)
                for n in range(N):
                    nc.sync.dma_start(out=vap[n, m], in_=out_t[n * 64:(n + 1) * 64, :])
    return nc


def permute_W(W: np.ndarray) -> np.ndarray:
    """(B, C, D, K, K) -> (128, M*D*KK): row n*64+b*2+clo holds W[b, 2m+clo, d, k, l]."""
    Wp = W.reshape(B, M, 2, D, KK).transpose(0, 2, 1, 3, 4).reshape(2 * B, M * D * KK)
    return np.ascontiguousarray(np.concatenate([Wp, Wp], axis=0))


def dup_poses(poses_shard: np.ndarray) -> np.ndarray:
    """(N, B, P, H, H) core shard -> (128, P*H*H): row n*64+b*2+clo = poses[n, b]."""
    flat = poses_shard.reshape(N, B, 1, P * H * H)
    return np.ascontiguousarray(np.broadcast_to(flat, (N, B, 2, P * H * H))
                                .reshape(NPART, P * H * H))


_cached_nc = None


def _get_nc():
    global _cached_nc
    if _cached_nc is None:
        nc = bacc.Bacc("TRN2", target_bir_lowering=False)
        _build(nc)
        nc.compile()
        _cached_nc = nc
    return _cached_nc


def run_spmd(poses: np.ndarray, W: np.ndarray, **spmd_kwargs):
    """Shard, run on 8 cores, gather. Returns (V_full, BassKernelResults)."""
    poses = np.ascontiguousarray(np.asarray(poses, dtype=np.float32))
    W = np.ascontiguousarray(np.asarray(W, dtype=np.float32))
    assert poses.shape == (NTOT, B, P, H, H), poses.shape
    assert W.shape == (B, C, D, K, K), W.shape
    Wp = permute_W(W)
    nc = _get_nc()
    in_maps = [{"poses": dup_poses(poses[i * N:(i + 1) * N]), "W": Wp}
               for i in range(NCORES)]
    res = bass_utils.run_bass_kernel_spmd(nc, in_maps, core_ids=list(range(NCORES)),
                                          **spmd_kwargs)
    V = np.concatenate([r["V"] for r in res.results], axis=0)
    return V, res


def kernel(poses: np.ndarray, W: np.ndarray) -> np.ndarray:
    import time
    last_err = None
    for attempt in range(3):
        try:
            V, _ = run_spmd(poses, W)
            return V
        except Exception as e:  # transient NRT/axon device errors: poke + retry
            last_err = e
            time.sleep(2.0)
            try:
                import jax, jax.numpy as jnp
                jnp.sum(jnp.ones((8, 8))).block_until_ready()
            except Exception:
                pass
    raise last_err
